# revision 1
# baseline (speedup 1.0000x reference)
"""Trainium2 Bass kernel for CustomStrainEnergyLoss.

Math (d = y_pred - y_true, f = clipped fracture_idx):
    pred_int_b - true_int_b = masked_trapz(d)                 (linearity)
    t_b  = sum_j 0.5*dx_j*(d_{b,j} + d_{b,j+1}) * [j < f_b]
    out  = mean_b(t_b^2)

For the uniform grid (x_values = arange, always true for this problem), with
m1 = [i<f], m2 = [i<=f] and m2 - m1 = [i==f]:
    sum_j (d_j + d_{j+1})*m1_j = sum_i d_i*m1_i + sum_i d_i*m2_i - d_0
                               = 2*sum_i d_i*[i<f] + (d_f - d_0)
so the device does just TWO DVE passes per element (memory-bound at the
~358 GB/s per-core HBM limit):
    d = yp - yt                          (tensor_sub)
    A = sum((iota < f) * d)              (scalar_tensor_tensor, fused accum)
then S = 2A + hcol per row ([128,1] ops), out = S^2.  hcol = d_f - d_0 is an
O(B) host-side gather. The 0.5*dx weight folds into a host-side scalar on the
final mean. A general path (non-uniform dx) multiplies the trapezoid segments
by a replicated 0.5*dx row on device instead.

Sharding: pure data parallel, 512 rows per core across 8 cores, y_pred on the
sync-engine HWDGE ring and y_true on the scalar-engine ring (alternating per
chunk); per-core row results [128, 4] are squared on device, summed on host.

This neuronx-cc build rejects instructions with >1 sync wait, so
_split_excess_waits moves extra waits onto same-engine NoOps post-schedule.
"""

import numpy as np

from concourse import bass
import concourse.mybir as mybir
from concourse.tile import TileContext
from concourse.bass_utils import run_bass_kernel_spmd

B, N = 4096, 8192
NCORES = 8
BS = B // NCORES          # 512 rows per core
P = 128                   # partitions
RT = BS // P              # 4 row-tiles per core
K = 2048                  # column chunk
NCH = N // K              # 4 chunks

_nc_cache = {}


def _split_excess_waits(nc, maxw: int = 1):
    """Workaround for this neuronx-cc build: walrus codegen rejects any
    instruction carrying more than one sync wait ("Too many sync wait
    commands" in setupSyncWait). Move extra waits onto same-engine NoOps
    inserted immediately before the instruction (sequencer executes them in
    order, so semantics are unchanged)."""
    for b in nc.main_func.blocks:
        newlist = []
        for ins in b.instructions:
            si = ins.sync_info
            ow = list(si.on_wait) if si else []
            if len(ow) > maxw:
                extra, keep = ow[:len(ow) - maxw], ow[len(ow) - maxw:]
                for i in range(0, len(extra), maxw):
                    nop = mybir.InstNoOp(
                        name=nc.get_next_instruction_name(), ins=[], outs=[])
                    nop.engine = ins.engine
                    nop.sync_info = mybir.SyncInfo(
                        on_wait=list(extra[i:i + maxw]), on_update=[])
                    nc.register_instruction(nop)
                    newlist.append(nop)
                ins.sync_info = mybir.SyncInfo(
                    on_wait=list(keep), on_update=list(si.on_update))
            newlist.append(ins)
        b.instructions[:] = newlist
    return nc


def build_nc_v2(reps: int = 1, io_bufs: int = 3, cmp_bufs: int = 2):
    """Uniform-dx fast path.

    S_b = sum_i d_i*[i<f_b] + sum_i d_i*[i<=f_b] - d_0   (all over full rows)
    Per [128, 4096] chunk: one tensor_sub + two fused STT mask-reduces.
    2 MiB DMA loads, y_pred on the sync HWDGE ring, y_true on the scalar ring.
    """
    f32 = mybir.dt.float32
    K2 = 4096
    NCH2 = N // K2  # 2
    nc = bass.Bass()
    yp = nc.declare_dram_parameter("yp", [BS, N], f32, isOutput=False)
    yt = nc.declare_dram_parameter("yt", [BS, N], f32, isOutput=False)
    fcl = nc.declare_dram_parameter("fcl", [BS, 1], f32, isOutput=False)
    o_sq = nc.declare_dram_parameter("o_sq", [P, RT], f32, isOutput=True)

    with TileContext(nc) as tc:
        with tc.tile_pool(name="pio", bufs=io_bufs) as pio, \
             tc.tile_pool(name="pcmp", bufs=cmp_bufs) as pc, \
             tc.tile_pool(name="pq", bufs=1) as pq, \
             tc.tile_pool(name="pers", bufs=1) as pp:
            iotas = []
            for c in range(NCH2):
                it = pp.tile([P, K2], f32, tag=f"iota{c}")
                nc.gpsimd.iota(
                    it, pattern=[[1, K2]], base=c * K2, channel_multiplier=0,
                    allow_small_or_imprecise_dtypes=True,
                )
                iotas.append(it)
            outt = pp.tile([P, RT], f32, tag="outt")

            for _rep in range(reps):
                for rt in range(RT):
                    r0 = rt * P
                    fcol = pc.tile([P, 1], f32, tag="fcol")
                    nc.sync.dma_start(out=fcol, in_=fcl[r0:r0 + P, :])
                    pab = pc.tile([P, 2 * NCH2], f32, tag="pab")
                    d0 = pc.tile([P, 1], f32, tag="d0")
                    for c in range(NCH2):
                        c0 = c * K2
                        ypt = pio.tile([P, K2], f32, tag="ypt")
                        ytt = pio.tile([P, K2], f32, tag="ytt")
                        nc.sync.dma_start(out=ypt, in_=yp[r0:r0 + P, c0:c0 + K2])
                        nc.scalar.dma_start(out=ytt, in_=yt[r0:r0 + P, c0:c0 + K2])
                        d = pc.tile([P, K2], f32, tag="d")
                        nc.vector.tensor_sub(out=d, in0=ypt, in1=ytt)
                        if c == 0:
                            nc.vector.tensor_copy(out=d0, in_=d[:, 0:1])
                        q = pq.tile([P, K2], f32, tag="q")
                        nc.vector.scalar_tensor_tensor(
                            out=q, in0=iotas[c], scalar=fcol, in1=d,
                            op0=mybir.AluOpType.is_lt, op1=mybir.AluOpType.mult,
                            accum_out=pab[:, c:c + 1],
                        )
                        nc.vector.scalar_tensor_tensor(
                            out=q, in0=iotas[c], scalar=fcol, in1=d,
                            op0=mybir.AluOpType.is_le, op1=mybir.AluOpType.mult,
                            accum_out=pab[:, NCH2 + c:NCH2 + c + 1],
                        )
                    ssum = pc.tile([P, 1], f32, tag="ssum")
                    nc.vector.tensor_reduce(
                        out=ssum, in_=pab, axis=mybir.AxisListType.X, op=mybir.AluOpType.add
                    )
                    st = pc.tile([P, 1], f32, tag="st")
                    nc.vector.tensor_sub(out=st, in0=ssum, in1=d0)
                    nc.vector.tensor_mul(out=outt[:, rt:rt + 1], in0=st, in1=st)
            nc.sync.dma_start(out=o_sq[:, :], in_=outt[:, :])
    return _split_excess_waits(nc)


def build_nc_v3(reps: int = 1, io_bufs: int = 3, cmp_bufs: int = 2,
                chunk_k: int = 4096, d_bufs: int = 2, batched_fh: bool = True,
                alt_rings: bool = False):
    """Uniform-dx fast path, 2 DVE passes per element.

    Identity: with m1 = [i<f], m2 = [i<=f],  m2 - m1 = [i==f], so
        S_b = sum_i d_i*m1 + sum_i d_i*m2 - d_0 = 2*sum_i d_i*[i<f] + (d_f - d_0).
    The host supplies hcol = d_f - d_0 per row (an O(B) gather); the device
    does d = yp - yt and ONE fused mask-reduce per chunk, then
    S = 2*A + hcol, out = S^2.
    """
    f32 = mybir.dt.float32
    K2 = chunk_k
    NCH2 = N // K2
    nc = bass.Bass()
    yp = nc.declare_dram_parameter("yp", [BS, N], f32, isOutput=False)
    yt = nc.declare_dram_parameter("yt", [BS, N], f32, isOutput=False)
    fcl = nc.declare_dram_parameter("fcl", [BS, 1], f32, isOutput=False)
    hcl = nc.declare_dram_parameter("hcl", [BS, 1], f32, isOutput=False)
    o_sq = nc.declare_dram_parameter("o_sq", [P, RT], f32, isOutput=True)
    # [512,1] viewed as [128, RT]: column rt holds rows rt*128..rt*128+127
    fview = fcl.rearrange("(rt p) one -> p (rt one)", p=P)
    hview = hcl.rearrange("(rt p) one -> p (rt one)", p=P)

    with TileContext(nc) as tc:
        with tc.tile_pool(name="pio", bufs=io_bufs) as pio, \
             tc.tile_pool(name="pcmp", bufs=cmp_bufs) as pc, \
             tc.tile_pool(name="pd", bufs=d_bufs) as pd, \
             tc.tile_pool(name="pq", bufs=1) as pq, \
             tc.tile_pool(name="pers", bufs=1) as pp:
            iotas = []
            for c in range(NCH2):
                it = pp.tile([P, K2], f32, tag=f"iota{c}")
                nc.gpsimd.iota(
                    it, pattern=[[1, K2]], base=c * K2, channel_multiplier=0,
                    allow_small_or_imprecise_dtypes=True,
                )
                iotas.append(it)
            outt = pp.tile([P, RT], f32, tag="outt")

            for _rep in range(reps):
                if batched_fh:
                    fcol4 = pc.tile([P, RT], f32, tag="fcol4")
                    nc.sync.dma_start(out=fcol4, in_=fview)
                    hcol4 = pc.tile([P, RT], f32, tag="hcol4")
                    nc.sync.dma_start(out=hcol4, in_=hview)
                for rt in range(RT):
                    r0 = rt * P
                    if not batched_fh:
                        fcol4 = pc.tile([P, RT], f32, tag="fcol4")
                        nc.sync.dma_start(out=fcol4[:, rt:rt + 1], in_=fcl[r0:r0 + P, :])
                        hcol4 = pc.tile([P, RT], f32, tag="hcol4")
                        nc.sync.dma_start(out=hcol4[:, rt:rt + 1], in_=hcl[r0:r0 + P, :])
                    pab = pc.tile([P, NCH2], f32, tag="pab")
                    for c in range(NCH2):
                        c0 = c * K2
                        ypt = pio.tile([P, K2], f32, tag="ypt")
                        ytt = pio.tile([P, K2], f32, tag="ytt")
                        e0, e1 = (nc.sync, nc.scalar)
                        if alt_rings and (rt * NCH2 + c) % 2 == 1:
                            e0, e1 = (nc.scalar, nc.sync)
                        e0.dma_start(out=ypt, in_=yp[r0:r0 + P, c0:c0 + K2])
                        e1.dma_start(out=ytt, in_=yt[r0:r0 + P, c0:c0 + K2])
                        d = pd.tile([P, K2], f32, tag="d")
                        nc.vector.tensor_sub(out=d, in0=ypt, in1=ytt)
                        q = pq.tile([P, K2], f32, tag="q")
                        nc.vector.scalar_tensor_tensor(
                            out=q, in0=iotas[c], scalar=fcol4[:, rt:rt + 1], in1=d,
                            op0=mybir.AluOpType.is_lt, op1=mybir.AluOpType.mult,
                            accum_out=pab[:, c:c + 1],
                        )
                    ssum = pc.tile([P, 1], f32, tag="ssum")
                    if NCH2 > 1:
                        nc.vector.tensor_reduce(
                            out=ssum, in_=pab, axis=mybir.AxisListType.X,
                            op=mybir.AluOpType.add,
                        )
                    else:
                        ssum = pab
                    st = pc.tile([P, 1], f32, tag="st")
                    nc.vector.scalar_tensor_tensor(
                        out=st, in0=ssum, scalar=2.0, in1=hcol4[:, rt:rt + 1],
                        op0=mybir.AluOpType.mult, op1=mybir.AluOpType.add,
                    )
                    nc.vector.tensor_mul(out=outt[:, rt:rt + 1], in0=st, in1=st)
            nc.sync.dma_start(out=o_sq[:, :], in_=outt[:, :])
    return _split_excess_waits(nc)


def build_nc(uniform: bool = True, reps: int = 1, io_bufs: int = 3, cmp_bufs: int = 2):
    f32 = mybir.dt.float32
    nc = bass.Bass()
    yp = nc.declare_dram_parameter("yp", [BS, N], f32, isOutput=False)
    yt = nc.declare_dram_parameter("yt", [BS, N], f32, isOutput=False)
    fcl = nc.declare_dram_parameter("fcl", [BS, 1], f32, isOutput=False)
    w = None
    if not uniform:
        w = nc.declare_dram_parameter("w", [P, N - 1], f32, isOutput=False)
    o_sq = nc.declare_dram_parameter("o_sq", [P, RT], f32, isOutput=True)

    with TileContext(nc) as tc:
        with tc.tile_pool(name="pio", bufs=io_bufs) as pio, \
             tc.tile_pool(name="pcmp", bufs=cmp_bufs) as pc, \
             tc.tile_pool(name="pers", bufs=1) as pp:
            # One-time: per-chunk f32 iota rows (values are exact ints < 2^24).
            iotas = []
            wts = []
            for c in range(NCH):
                seg = K if c < NCH - 1 else K - 1
                it = pp.tile([P, seg], f32, tag=f"iota{c}")
                nc.gpsimd.iota(
                    it, pattern=[[1, seg]], base=c * K, channel_multiplier=0,
                    allow_small_or_imprecise_dtypes=True,
                )
                iotas.append(it)
                if not uniform:
                    wt = pp.tile([P, seg], f32, tag=f"w{c}")
                    nc.sync.dma_start(out=wt, in_=w[:, c * K:c * K + seg])
                    wts.append(wt)
            outt = pp.tile([P, RT], f32, tag="outt")

            for _rep in range(reps):
                for rt in range(RT):
                    r0 = rt * P
                    fcol = pc.tile([P, 1], f32, tag="fcol")
                    nc.sync.dma_start(out=fcol, in_=fcl[r0:r0 + P, :])
                    p4 = pc.tile([P, NCH], f32, tag="p4")
                    for c in range(NCH):
                        lw = K + 1 if c < NCH - 1 else K   # load width
                        seg = lw - 1                       # segments
                        c0 = c * K
                        ypt = pio.tile([P, K + 1], f32, tag="ypt")
                        ytt = pio.tile([P, K + 1], f32, tag="ytt")
                        nc.sync.dma_start(out=ypt[:, :lw], in_=yp[r0:r0 + P, c0:c0 + lw])
                        nc.sync.dma_start(out=ytt[:, :lw], in_=yt[r0:r0 + P, c0:c0 + lw])
                        d = pc.tile([P, K + 1], f32, tag="d")
                        nc.vector.tensor_sub(out=d[:, :lw], in0=ypt[:, :lw], in1=ytt[:, :lw])
                        s = pc.tile([P, K], f32, tag="s")
                        nc.vector.tensor_add(out=s[:, :seg], in0=d[:, 0:seg], in1=d[:, 1:seg + 1])
                        src = s
                        if not uniform:
                            u = pc.tile([P, K], f32, tag="u")
                            nc.vector.tensor_mul(out=u[:, :seg], in0=s[:, :seg], in1=wts[c][:, :seg])
                            src = u
                        q = pc.tile([P, K], f32, tag="q")
                        nc.vector.scalar_tensor_tensor(
                            out=q[:, :seg], in0=iotas[c][:, :seg], scalar=fcol,
                            in1=src[:, :seg],
                            op0=mybir.AluOpType.is_lt, op1=mybir.AluOpType.mult,
                            accum_out=p4[:, c:c + 1],
                        )
                    st = pc.tile([P, 1], f32, tag="st")
                    nc.vector.tensor_reduce(
                        out=st, in_=p4, axis=mybir.AxisListType.X, op=mybir.AluOpType.add
                    )
                    nc.vector.tensor_mul(out=outt[:, rt:rt + 1], in0=st, in1=st)
            nc.sync.dma_start(out=o_sq[:, :], in_=outt[:, :])
    return _split_excess_waits(nc)


def make_in_maps(y_pred, y_true, x_values, fracture_idx):
    y_pred = np.ascontiguousarray(np.asarray(y_pred, dtype=np.float32))
    y_true = np.ascontiguousarray(np.asarray(y_true, dtype=np.float32))
    x = np.asarray(x_values, dtype=np.float32)
    idx = np.clip(np.asarray(fracture_idx).astype(np.int64), 0, N - 1)
    f = idx.astype(np.float32).reshape(B, 1)

    dx = np.diff(x)
    uniform = bool(np.all(dx == dx[0]))
    if uniform:
        scale = float(0.5 * dx[0]) ** 2 / B
    else:
        scale = 1.0 / B

    # hcl = d_f - d_0 per row (O(B) host gather; see build_nc_v3 docstring)
    rows = np.arange(B)
    d_f = y_pred[rows, idx] - y_true[rows, idx]
    d_0 = y_pred[:, 0] - y_true[:, 0]
    h = (d_f - d_0).astype(np.float32).reshape(B, 1)

    in_maps = []
    for c in range(NCORES):
        r0 = c * BS
        m = {
            "yp": y_pred[r0:r0 + BS],
            "yt": y_true[r0:r0 + BS],
            "fcl": np.ascontiguousarray(f[r0:r0 + BS]),
            "hcl": np.ascontiguousarray(h[r0:r0 + BS]),
        }
        if not uniform:
            wrow = (0.5 * dx).astype(np.float32)
            m["w"] = np.ascontiguousarray(np.broadcast_to(wrow, (P, N - 1)))
        in_maps.append(m)
    return in_maps, uniform, scale


def kernel(y_pred, y_true, x_values, fracture_idx):
    assert y_pred.shape == (B, N), y_pred.shape
    in_maps, uniform, scale = make_in_maps(y_pred, y_true, x_values, fracture_idx)
    key = ("main", uniform)
    if key not in _nc_cache:
        _nc_cache[key] = (
            build_nc_v3(io_bufs=3, d_bufs=1, chunk_k=4096, alt_rings=True)
            if uniform else build_nc(uniform=False)
        )
    nc = _nc_cache[key]
    res = None
    last_err = None
    for _attempt in range(3):
        try:
            res = run_bass_kernel_spmd(nc, in_maps, list(range(NCORES)))
            break
        except Exception as e:  # sporadic NRT_EXEC_UNIT_UNRECOVERABLE on this infra
            last_err = e
            try:
                import jax
                jax.clear_backends()
            except Exception:
                pass
    if res is None:
        raise last_err
    total = 0.0
    for c in range(NCORES):
        total += np.asarray(res.results[c]["o_sq"], dtype=np.float64).sum()
    return np.asarray(total * scale, dtype=np.float32)

